# revision 17
# baseline (speedup 1.0000x reference)
"""LogicLayer Trainium2 kernel: out = c0 + c1*x[:,ia] + c2*x[:,ib] + c3*x[:,ia]*x[:,ib]
with coef = softmax(weights) @ OP_COEFFS (softmax+projection computed on-device).

Sharding: out_dim (neuron) split across 8 cores, 2048 neurons each; x is
replicated, staged host-side pre-transposed to xT [in_dim, batch] fp16 so each
neuron's column reads are contiguous 4 KB rows.

Gather dedup: neurons are edges (ia[o], ib[o]) over column-vertices. The host
decomposes this multigraph into trails and packs them into 1024 "tracks"
(8 cores x 128 partitions), each holding 16 neurons arranged in chained
segments per a uniform template: consecutive neurons in a segment share a
column, so a length-L segment gathers L+1 columns instead of 2L. Orientation
swaps (a<->b) are absorbed by permuting each neuron's 16 op-weights host-side.
Columns are fetched with bulk SWDGE dma_gather calls on 2 queues.

Compute per chunk c (128 neurons x 2048 batch, fp16):
  ACT:  u = c3*b + c1
  PE :  w = diag(c2) @ b + diag(c0) @ ones   -> PSUM (fp32)
  DVE:  p = a * u ;  out = p + w             -> fp16, DMA'd out (SP/ACT queues)
Diagonal stationaries are built on DVE as mask*broadcast(ck) products.
"""
import sys

sys.path.insert(0, "/opt/trn_rl_repo")
from collections import defaultdict

import numpy as np

import concourse.bass as bass  # noqa: F401
import concourse.bacc as bacc
from concourse import mybir
from concourse.bass_utils import run_bass_kernel_spmd

_OP_COEFFS = np.array([
    [0., 0., 0., 0.], [0., 0., 0., 1.], [0., 1., 0., -1.], [0., 1., 0., 0.],
    [0., 0., 1., -1.], [0., 0., 1., 0.], [0., 1., 1., -2.], [0., 1., 1., -1.],
    [1., -1., -1., 1.], [1., -1., -1., 2.], [1., 0., -1., 0.], [1., 0., -1., 1.],
    [1., -1., 0., 0.], [1., -1., 0., 1.], [1., 0., 0., -1.], [1., 0., 0., 0.],
], dtype=np.float32)

# op permutation under a<->b swap: coeffs (c0,c1,c2,c3) -> (c0,c2,c1,c3)
_SWAP_PERM = np.array([0, 1, 4, 5, 2, 3, 6, 7, 8, 9, 12, 13, 10, 11, 14, 15])

BATCH, IN_DIM, OUT_DIM = 2048, 16384, 16384
NCORES = 8
N = OUT_DIM // NCORES        # 2048 neurons per core
B = BATCH
NCH = N // 128               # 16 chunks of 128 neurons
NQ = 8                       # pipeline stages
QC = NCH // NQ               # 2 chunks per stage
NTRACK = NCORES * 128
F32 = mybir.dt.float32
F16 = mybir.dt.float16
I16 = mybir.dt.int16
I32 = mybir.dt.int32
AX = mybir.AxisListType.X
IDENT = mybir.ActivationFunctionType.Identity
EXP = mybir.ActivationFunctionType.Exp
MULT = mybir.AluOpType.mult
ADD = mybir.AluOpType.add

_cached = {}


# ---------------------------------------------------------------- planner ---

def _euler_trails(idx_a, idx_b):
    """Decompose the neuron multigraph into trails (Euler-minimal-ish).
    Returns list of trails; each trail = list of (edge_id, u, v) walk steps."""
    n_edge = len(idx_a)
    adj = defaultdict(list)
    deg = defaultdict(int)
    for o in range(n_edge):
        u, v = int(idx_a[o]), int(idx_b[o])
        adj[u].append((o, v, False))
        deg[u] += 1
        if u != v:
            adj[v].append((o, u, False))
            deg[v] += 1
        else:
            deg[u] += 1
    seen = set()
    comps = []
    for s in adj:
        if s in seen:
            continue
        comp, stack = [], [s]
        seen.add(s)
        while stack:
            x = stack.pop()
            comp.append(x)
            for _, y, _ in adj[x]:
                if y not in seen:
                    seen.add(y)
                    stack.append(y)
        comps.append(comp)

    virt = len(idx_a)
    for comp in comps:
        odd = [v for v in comp if deg[v] % 2 == 1]
        for i in range(0, len(odd), 2):
            u, v = odd[i], odd[i + 1]
            adj[u].append((virt, v, True))
            adj[v].append((virt, u, True))
            virt += 1

    used = np.zeros(virt, dtype=bool)
    ptr = defaultdict(int)

    def circuit(start):
        stack = [(start, None)]
        out = []
        while stack:
            x, _ = stack[-1]
            lst = adj[x]
            p = ptr[x]
            while p < len(lst) and used[lst[p][0]]:
                p += 1
            ptr[x] = p
            if p < len(lst):
                eid, y, isv = lst[p]
                used[eid] = True
                stack.append((y, (eid, x, y, isv)))
            else:
                _, rec = stack.pop()
                if rec is not None:
                    out.append(rec)
        out.reverse()
        return out

    trails = []
    for comp in comps:
        start = None
        for v in comp:
            if any(not used[e[0]] for e in adj[v]):
                start = v
                break
        if start is None:
            continue
        circ = circuit(start)
        # rotate the closed circuit to begin at a virtual edge so the wrap
        # doesn't split one trail into two
        vpos = [i for i, (_, _, _, isv) in enumerate(circ) if isv]
        if vpos:
            k = vpos[0]
            circ = circ[k:] + circ[:k]
        cur = []
        for (eid, u, v, isv) in circ:
            if isv:
                if cur:
                    trails.append(cur)
                cur = []
            else:
                cur.append((eid, u, v))
        if cur:
            trails.append(cur)
    return trails


def _cut_trails(trails, demand):
    """Cut trails into pieces matching demand {length: count} exactly.
    Returns pieces list or None if the greedy fails."""
    import heapq
    demand = dict(demand)
    pieces = []
    rest = []
    for t in sorted(trails, key=len):
        L = len(t)
        if demand.get(L, 0) > 0:
            demand[L] -= 1
            pieces.append(t)
        else:
            rest.append(t)
    store = list(rest)
    heap = [(-len(t), i) for i, t in enumerate(store)]
    heapq.heapify(heap)

    def push(t):
        store.append(t)
        heapq.heappush(heap, (-len(t), len(store) - 1))

    for l in sorted(demand, reverse=True):
        while demand[l] > 0:
            if not heap or -heap[0][0] < l:
                return None
            negL, ti = heapq.heappop(heap)
            t = store[ti]
            demand[l] -= 1
            pieces.append(t[:l])
            if -negL > l:
                push(t[l:])
    while heap:
        _, ti = heapq.heappop(heap)
        if len(store[ti]) > 0:
            return None
    if any(v != 0 for v in demand.values()):
        return None
    return pieces


def _gen_templates(total=16, maxpart=12):
    results = []

    def rec(remaining, mx, cur):
        if remaining == 0:
            results.append(tuple(cur))
            return
        for p in range(min(mx, remaining), 0, -1):
            rec(remaining - p, p, cur + [p])

    rec(total, maxpart, [])
    results.sort(key=len)
    return results


def _plan(idx_a, idx_b):
    """Returns (segs, slot_cols[NTRACK,S], perm[NTRACK,NCH], swap[NTRACK,NCH])."""
    trails = _euler_trails(idx_a, idx_b)
    segs, pieces = None, None
    for tpl in _gen_templates():
        demand = defaultdict(int)
        for l in tpl:
            demand[l] += NTRACK
        got = _cut_trails(trails, demand)
        if got is not None:
            # ascending: big slot-groups land during ramp (overlapped with the
            # coef chain), the final stage needs only a small fresh group
            segs, pieces = sorted(tpl), got
            break
    assert segs is not None, "planner failed (singleton template should be feasible)"

    by_len = defaultdict(list)
    for p in pieces:
        by_len[len(p)].append(p)

    S = NCH + len(segs)
    slot_cols = np.zeros((NTRACK, S), dtype=np.int32)
    perm = np.zeros((NTRACK, NCH), dtype=np.int64)
    swap = np.zeros((NTRACK, NCH), dtype=bool)
    for t in range(NTRACK):
        base, chunk = 0, 0
        for l in segs:
            piece = by_len[l].pop()
            for i, (eid, u, v) in enumerate(piece):
                slot_cols[t, base + i] = u
                perm[t, chunk] = eid
                swap[t, chunk] = not (u == idx_a[eid] and v == idx_b[eid])
                chunk += 1
            slot_cols[t, base + l] = piece[-1][2]
            base += l + 1
        assert chunk == NCH and base == S
    return segs, slot_cols, perm, swap


def _seg_slots(segs):
    """Per-chunk (a_slot, b_slot) for the template."""
    sa, sb, base = [], [], 0
    for l in segs:
        for i in range(l):
            sa.append(base + i)
            sb.append(base + i + 1)
        base += l + 1
    return sa, sb, base


# ----------------------------------------------------------------- device ---

NPOOL = 5  # trailing chunks final-combined on Pool (after gather issue)


def build_nc(segs):
    sa, sb, S = _seg_slots(segs)
    # gather groups: stage q needs slots <= smax(q); group q = new slots
    smax = [max(sb[2 * q], sb[2 * q + 1]) for q in range(NQ)]
    groups = []
    lo = 0
    for q in range(NQ):
        hi = smax[q] + 1
        groups.append((lo, hi))
        lo = hi

    nc = bacc.Bacc("TRN2", target_bir_lowering=False, num_swdge_queues=1)
    xt = nc.declare_dram_parameter("xt", [IN_DIM, B], F16, isOutput=False)
    ist_in = nc.declare_dram_parameter("ist", [128, S], I32, isOutput=False)
    wc_in = nc.declare_dram_parameter("wc", [128, NCH * 16], F32, isOutput=False)
    opc_in = nc.declare_dram_parameter("opc", [128, 4 * NCH * 16], F32, isOutput=False)
    out = nc.declare_dram_parameter("out", [N, B], F16, isOutput=True)
    # out is [N, B]; view as [128, NCH, B] partition-major: DRAM row p*NCH+c.
    # Each partition's NCH rows are contiguous -> 8 KB store packets per stage.
    out_pcb = out.ap().rearrange("(p c) b -> p c b", p=128)

    from contextlib import ExitStack
    es = ExitStack()
    sb_ = lambda n, shape, dt=F32: es.enter_context(nc.sbuf_tensor(n, shape, dt))
    sem = lambda n: es.enter_context(nc.semaphore(n))
    g = sb_("g", [128, S, B], F16)
    ist = sb_("istb", [128, S], I32)
    wt = sb_("wt", [128, NCH * 16])
    opcf = sb_("opcf", [128, 4, NCH * 16])
    m3 = sb_("m3", [128, NCH * 16])
    ssum = sb_("ssum", [128, NCH]); rinv = sb_("rinv", [128, NCH])
    ckn = sb_("ckn", [128, NCH]); ck = sb_("ck", [128, 4, NCH])
    uf = sb_("uf", [128, 3, QC, B], F16)
    pf = sb_("pf", [128, 3, QC, B], F16)
    ob = sb_("ob", [128, 3, QC, B], F16)
    wf = sb_("wf", [128, NPOOL, B], F16)

    ldi = sem("ldi"); ldwt = sem("ldwt"); ldop = sem("ldop")
    gsg = [sem(f"gs{q}") for q in range(NQ)]
    cfA = sem("cfA"); cfE = sem("cfE"); ua = sem("ua")
    adV = sem("adV"); adP = sem("adP")
    pmd = sem("pmd"); pwE = sem("pwE"); ccs = sem("ccs")
    ods = [sem(f"od{s}") for s in range(NQ)]
    NDVE = NCH - NPOOL          # chunks 0..NDVE-1 final-combined on DVE

    def ob_free_wait(eng, q):
        # ob[q%3] free once the store of stage q-3 completed
        if q >= 3:
            eng.wait_ge(ods[q - 3], 16)

    def wait_ad(eng, n):
        # wait until the first n chunk-combines are done (DVE prefix 0..NDVE-1,
        # Pool suffix NDVE..); split because the two streams complete unordered
        if n <= 0:
            return
        eng.wait_ge(adV, min(n, NDVE))
        if n > NDVE:
            eng.wait_ge(adP, n - NDVE)

    with es, nc.Block() as block:

        @block.sync
        def _(sync):
            sync.dma_start(ist[:], ist_in[:]).then_inc(ldi, 16)
            sync.dma_start(wt[:], wc_in[:]).then_inc(ldwt, 16)
            sync.dma_start(
                opcf.ap().rearrange("p a b -> p (a b)"), opc_in[:]
            ).then_inc(ldop, 16)
            for q in range(0, NQ, 2):   # even stages
                wait_ad(sync, 2 * (q + 1))
                sync.dma_start(
                    out_pcb[:, 2 * q:2 * q + 2, :], ob[:, q % 3]
                ).then_inc(ods[q], 16)

        @block.gpsimd
        def _(gp):
            gp.wait_ge(ldi, 16)
            for q, (lo, hi) in enumerate(groups):
                for s in range(lo, hi):
                    gp.indirect_dma_start(
                        out=g[:, s, :], out_offset=None, in_=xt[:],
                        in_offset=bass.IndirectOffsetOnAxis(
                            ap=ist[:, s:s + 1], axis=0),
                    ).then_inc(gsg[q], 16)
            # trailing chunks: out = p + w, with w = c2*b + c0 from ACT
            for i, c in enumerate(range(NDVE, NCH)):
                q = c // QC
                j = c % QC
                gp.wait_ge(pwE, i + 1)          # ACT's w ready
                gp.wait_ge(pmd, c + 1)
                ob_free_wait(gp, q)
                gp.tensor_add(                  # out = p + w
                    ob[:, q % 3, j, :], pf[:, q % 3, j, :], wf[:, i, :],
                ).then_inc(adP, 1)

        @block.scalar
        def _(act):
            act.wait_ge(ldwt, 16)
            act.activation(wt[:], wt[:], EXP).then_inc(cfA, 1)
            act.wait_ge(cfE, 1)
            for q in range(NQ):
                if q >= 2 and (q - 1) % 2 == 1:   # store odd stage q-1
                    wait_ad(act, 2 * q)
                    act.dma_start(
                        out_pcb[:, 2 * (q - 1):2 * (q - 1) + 2, :],
                        ob[:, (q - 1) % 3],
                    ).then_inc(ods[q - 1], 16)
                act.wait_ge(gsg[q], 16 * (groups[q][1] - groups[q][0]))
                if q >= 3:
                    wait_ad(act, 2 * (q - 2))      # uf[q%3] free
                for j in range(QC):
                    c = q * QC + j
                    act.activation(                  # u = c3*b + c1
                        uf[:, q % 3, j, :], g[:, sb[c], :], IDENT,
                        bias=ck[:, 1, c:c + 1], scale=ck[:, 3, c:c + 1],
                    ).then_inc(ua, 1)
                for j in range(QC):
                    c = q * QC + j
                    if c >= NDVE:                    # w = c2*b + c0 for Pool
                        act.activation(
                            wf[:, c - NDVE, :], g[:, sb[c], :], IDENT,
                            bias=ck[:, 0, c:c + 1], scale=ck[:, 2, c:c + 1],
                        ).then_inc(pwE, 1)
            wait_ad(act, 2 * NQ)
            act.dma_start(
                out_pcb[:, 2 * (NQ - 1):2 * NQ, :], ob[:, (NQ - 1) % 3]
            ).then_inc(ods[NQ - 1], 16)

        @block.vector
        def _(vec):
            nedge = [0]

            def edge(inst):
                nedge[0] += 1
                inst.then_inc(ccs, 1)
                vec.wait_ge(ccs, nedge[0])

            vec.wait_ge(cfA, 1)
            e3 = wt.ap().rearrange("p (a b) -> p a b", b=16)
            m3r = m3.ap().rearrange("p (a b) -> p a b", b=16)
            edge(vec.reduce_sum(ssum[:], e3, axis=AX))
            edge(vec.reciprocal(rinv[:], ssum[:]))
            vec.wait_ge(ldop, 16)
            for k in range(4):
                o3c = opcf[:, k, :].rearrange("p (a b) -> p a b", b=16)
                edge(vec.tensor_mul(m3r, e3, o3c))
                edge(vec.reduce_sum(ckn[:], m3r, axis=AX))
                edge(vec.tensor_mul(ck[:, k, :], ckn[:], rinv[:]))
            vec.memset(m3[:, 0:1], 0.0).then_inc(cfE, 1)  # fence: ck committed
            for q in range(NQ):
                vec.wait_ge(ua, QC * (q + 1))
                for j in range(QC):
                    c = q * QC + j
                    vec.tensor_mul(                 # p = a*u
                        pf[:, q % 3, j, :], g[:, sa[c], :], uf[:, q % 3, j, :],
                    ).then_inc(pmd, 1)
                for j in range(QC):
                    c = q * QC + j
                    if c >= NDVE:
                        continue                    # Pool handles trailing chunks
                    vec.wait_ge(pmd, c + 1)         # own mul drained (DVE RAW)
                    if j == 0:
                        ob_free_wait(vec, q)
                    vec.affine_then_add(            # out = (b*c2 + c0) + p
                        ob[:, q % 3, j, :], g[:, sb[c], :],
                        pf[:, q % 3, j, :],
                        ck[:, 2, c:c + 1], ck[:, 0, c:c + 1],
                    ).then_inc(adV, 1)

    nc.compile()
    return nc


def kernel(x, idx_a, idx_b, weights, trace=False):
    x = np.asarray(x, dtype=np.float32)
    weights = np.asarray(weights, dtype=np.float32)
    idx_a = np.asarray(idx_a).astype(np.int64)
    idx_b = np.asarray(idx_b).astype(np.int64)
    assert x.shape == (BATCH, IN_DIM) and weights.shape == (OUT_DIM, 16)

    key = (idx_a.tobytes(), idx_b.tobytes())
    if _cached.get("plan_key") != key:
        segs, slot_cols, perm, swap = _plan(idx_a, idx_b)
        _cached.update(plan_key=key, segs=segs, slot_cols=slot_cols,
                       perm=perm, swap=swap)
        if _cached.get("nc_segs") != tuple(segs):
            _cached["nc"] = build_nc(segs)
            _cached["nc_segs"] = tuple(segs)
    nc = _cached["nc"]
    segs, slot_cols = _cached["segs"], _cached["slot_cols"]
    perm, swap = _cached["perm"], _cached["swap"]

    xt = np.ascontiguousarray(x.astype(np.float16).T)  # [IN_DIM, B] fp16
    opc_row = np.repeat(_OP_COEFFS.T[:, None, :], NCH, axis=1).reshape(4 * NCH * 16)
    opc = np.ascontiguousarray(
        np.broadcast_to(opc_row[None, :], (128, 4 * NCH * 16))
    ).astype(np.float32)

    # effective per-neuron weights with a<->b swap permutation applied
    w_eff_all = np.where(swap.reshape(-1)[:, None],
                         weights[perm.reshape(-1)][:, _SWAP_PERM],
                         weights[perm.reshape(-1)])          # [NTRACK*NCH, 16]
    w_eff_all = w_eff_all.reshape(NCORES, 128, NCH, 16)

    in_maps = []
    for kcore in range(NCORES):
        tr = slice(kcore * 128, (kcore + 1) * 128)
        wc = np.ascontiguousarray(
            w_eff_all[kcore].reshape(128, NCH * 16)).astype(np.float32)
        in_maps.append({
            "xt": xt, "wc": wc, "opc": opc,
            "ist": np.ascontiguousarray(slot_cols[tr]).astype(np.int32),
        })
    res = run_bass_kernel_spmd(nc, in_maps, core_ids=list(range(NCORES)), trace=trace)

    buf = np.empty((OUT_DIM, B), dtype=np.float16)
    for kcore in range(NCORES):
        r = res.results[kcore]["out"].reshape(128, NCH, B)
        buf[perm[kcore * 128:(kcore + 1) * 128]] = r
    out = buf.T.astype(np.float32)
    kernel.last_exec_time_ns = res.exec_time_ns
    return out


kernel.last_exec_time_ns = None


# revision 18
# speedup vs baseline: 1.1278x; 1.1278x over previous
"""LogicLayer Trainium2 kernel: out = c0 + c1*x[:,ia] + c2*x[:,ib] + c3*x[:,ia]*x[:,ib]
with coef = softmax(weights) @ OP_COEFFS (softmax+projection computed on-device).

Sharding: out_dim (neuron) split across 8 cores, 2048 neurons each; x is
replicated, staged host-side pre-transposed to xT [in_dim, batch] fp16 so each
neuron's column reads are contiguous 4 KB rows.

Gather dedup: neurons are edges (ia[o], ib[o]) over column-vertices. The host
decomposes this multigraph into trails and packs them into 1024 "tracks"
(8 cores x 128 partitions), each holding 16 neurons arranged in chained
segments per a uniform template: consecutive neurons in a segment share a
column, so a length-L segment gathers L+1 columns instead of 2L. Orientation
swaps (a<->b) are absorbed by permuting each neuron's 16 op-weights host-side.
Columns are fetched with bulk SWDGE dma_gather calls on 2 queues.

Compute per chunk c (128 neurons x 2048 batch, fp16):
  ACT:  u = c3*b + c1
  PE :  w = diag(c2) @ b + diag(c0) @ ones   -> PSUM (fp32)
  DVE:  p = a * u ;  out = p + w             -> fp16, DMA'd out (SP/ACT queues)
Diagonal stationaries are built on DVE as mask*broadcast(ck) products.
"""
import sys

sys.path.insert(0, "/opt/trn_rl_repo")
from collections import defaultdict

import numpy as np

import concourse.bass as bass  # noqa: F401
import concourse.bacc as bacc
from concourse import mybir
from concourse.bass_utils import run_bass_kernel_spmd

_OP_COEFFS = np.array([
    [0., 0., 0., 0.], [0., 0., 0., 1.], [0., 1., 0., -1.], [0., 1., 0., 0.],
    [0., 0., 1., -1.], [0., 0., 1., 0.], [0., 1., 1., -2.], [0., 1., 1., -1.],
    [1., -1., -1., 1.], [1., -1., -1., 2.], [1., 0., -1., 0.], [1., 0., -1., 1.],
    [1., -1., 0., 0.], [1., -1., 0., 1.], [1., 0., 0., -1.], [1., 0., 0., 0.],
], dtype=np.float32)

# op permutation under a<->b swap: coeffs (c0,c1,c2,c3) -> (c0,c2,c1,c3)
_SWAP_PERM = np.array([0, 1, 4, 5, 2, 3, 6, 7, 8, 9, 12, 13, 10, 11, 14, 15])

BATCH, IN_DIM, OUT_DIM = 2048, 16384, 16384
NCORES = 8
N = OUT_DIM // NCORES        # 2048 neurons per core
B = BATCH
NCH = N // 128               # 16 chunks of 128 neurons
NQ = 8                       # pipeline stages
QC = NCH // NQ               # 2 chunks per stage
NTRACK = NCORES * 128
F32 = mybir.dt.float32
F16 = mybir.dt.float16
I16 = mybir.dt.int16
I32 = mybir.dt.int32
AX = mybir.AxisListType.X
IDENT = mybir.ActivationFunctionType.Identity
EXP = mybir.ActivationFunctionType.Exp
MULT = mybir.AluOpType.mult
ADD = mybir.AluOpType.add

_cached = {}


# ---------------------------------------------------------------- planner ---

def _euler_trails(idx_a, idx_b):
    """Decompose the neuron multigraph into trails (Euler-minimal-ish).
    Returns list of trails; each trail = list of (edge_id, u, v) walk steps."""
    n_edge = len(idx_a)
    adj = defaultdict(list)
    deg = defaultdict(int)
    for o in range(n_edge):
        u, v = int(idx_a[o]), int(idx_b[o])
        adj[u].append((o, v, False))
        deg[u] += 1
        if u != v:
            adj[v].append((o, u, False))
            deg[v] += 1
        else:
            deg[u] += 1
    seen = set()
    comps = []
    for s in adj:
        if s in seen:
            continue
        comp, stack = [], [s]
        seen.add(s)
        while stack:
            x = stack.pop()
            comp.append(x)
            for _, y, _ in adj[x]:
                if y not in seen:
                    seen.add(y)
                    stack.append(y)
        comps.append(comp)

    virt = len(idx_a)
    for comp in comps:
        odd = [v for v in comp if deg[v] % 2 == 1]
        for i in range(0, len(odd), 2):
            u, v = odd[i], odd[i + 1]
            adj[u].append((virt, v, True))
            adj[v].append((virt, u, True))
            virt += 1

    used = np.zeros(virt, dtype=bool)
    ptr = defaultdict(int)

    def circuit(start):
        stack = [(start, None)]
        out = []
        while stack:
            x, _ = stack[-1]
            lst = adj[x]
            p = ptr[x]
            while p < len(lst) and used[lst[p][0]]:
                p += 1
            ptr[x] = p
            if p < len(lst):
                eid, y, isv = lst[p]
                used[eid] = True
                stack.append((y, (eid, x, y, isv)))
            else:
                _, rec = stack.pop()
                if rec is not None:
                    out.append(rec)
        out.reverse()
        return out

    trails = []
    for comp in comps:
        start = None
        for v in comp:
            if any(not used[e[0]] for e in adj[v]):
                start = v
                break
        if start is None:
            continue
        circ = circuit(start)
        # rotate the closed circuit to begin at a virtual edge so the wrap
        # doesn't split one trail into two
        vpos = [i for i, (_, _, _, isv) in enumerate(circ) if isv]
        if vpos:
            k = vpos[0]
            circ = circ[k:] + circ[:k]
        cur = []
        for (eid, u, v, isv) in circ:
            if isv:
                if cur:
                    trails.append(cur)
                cur = []
            else:
                cur.append((eid, u, v))
        if cur:
            trails.append(cur)
    return trails


def _cut_trails(trails, demand):
    """Cut trails into pieces matching demand {length: count} exactly.
    Returns pieces list or None if the greedy fails."""
    import heapq
    demand = dict(demand)
    pieces = []
    rest = []
    for t in sorted(trails, key=len):
        L = len(t)
        if demand.get(L, 0) > 0:
            demand[L] -= 1
            pieces.append(t)
        else:
            rest.append(t)
    store = list(rest)
    heap = [(-len(t), i) for i, t in enumerate(store)]
    heapq.heapify(heap)

    def push(t):
        store.append(t)
        heapq.heappush(heap, (-len(t), len(store) - 1))

    for l in sorted(demand, reverse=True):
        while demand[l] > 0:
            if not heap or -heap[0][0] < l:
                return None
            negL, ti = heapq.heappop(heap)
            t = store[ti]
            demand[l] -= 1
            pieces.append(t[:l])
            if -negL > l:
                push(t[l:])
    while heap:
        _, ti = heapq.heappop(heap)
        if len(store[ti]) > 0:
            return None
    if any(v != 0 for v in demand.values()):
        return None
    return pieces


def _gen_templates(total=16, maxpart=12):
    results = []

    def rec(remaining, mx, cur):
        if remaining == 0:
            results.append(tuple(cur))
            return
        for p in range(min(mx, remaining), 0, -1):
            rec(remaining - p, p, cur + [p])

    rec(total, maxpart, [])
    results.sort(key=len)
    return results


def _plan(idx_a, idx_b):
    """Returns (segs, slot_cols[NTRACK,S], perm[NTRACK,NCH], swap[NTRACK,NCH])."""
    trails = _euler_trails(idx_a, idx_b)
    segs, pieces = None, None
    for tpl in _gen_templates():
        demand = defaultdict(int)
        for l in tpl:
            demand[l] += NTRACK
        got = _cut_trails(trails, demand)
        if got is not None:
            # ascending: big slot-groups land during ramp (overlapped with the
            # coef chain), the final stage needs only a small fresh group
            segs, pieces = sorted(tpl), got
            break
    assert segs is not None, "planner failed (singleton template should be feasible)"

    by_len = defaultdict(list)
    for p in pieces:
        by_len[len(p)].append(p)

    S = NCH + len(segs)
    slot_cols = np.zeros((NTRACK, S), dtype=np.int32)
    perm = np.zeros((NTRACK, NCH), dtype=np.int64)
    swap = np.zeros((NTRACK, NCH), dtype=bool)
    for t in range(NTRACK):
        base, chunk = 0, 0
        for l in segs:
            piece = by_len[l].pop()
            for i, (eid, u, v) in enumerate(piece):
                slot_cols[t, base + i] = u
                perm[t, chunk] = eid
                swap[t, chunk] = not (u == idx_a[eid] and v == idx_b[eid])
                chunk += 1
            slot_cols[t, base + l] = piece[-1][2]
            base += l + 1
        assert chunk == NCH and base == S
    return segs, slot_cols, perm, swap


def _seg_slots(segs):
    """Per-chunk (a_slot, b_slot) for the template."""
    sa, sb, base = [], [], 0
    for l in segs:
        for i in range(l):
            sa.append(base + i)
            sb.append(base + i + 1)
        base += l + 1
    return sa, sb, base


# ----------------------------------------------------------------- device ---

NPOOL = 5  # trailing chunks final-combined on Pool (after gather issue)


def build_nc(segs):
    sa, sb, S = _seg_slots(segs)
    # gather groups: stage q needs slots <= smax(q); group q = new slots
    smax = [max(sb[2 * q], sb[2 * q + 1]) for q in range(NQ)]
    groups = []
    lo = 0
    for q in range(NQ):
        hi = smax[q] + 1
        groups.append((lo, hi))
        lo = hi

    nc = bacc.Bacc("TRN2", target_bir_lowering=False, num_swdge_queues=1)
    xt = nc.declare_dram_parameter("xt", [IN_DIM, B], F16, isOutput=False)
    ist_in = nc.declare_dram_parameter("ist", [128, S], I32, isOutput=False)
    wc_in = nc.declare_dram_parameter("wc", [128, NCH * 16], F32, isOutput=False)
    opc_in = nc.declare_dram_parameter("opc", [128, 4 * NCH * 16], F32, isOutput=False)
    out = nc.declare_dram_parameter("out", [N, B], F16, isOutput=True)
    # out is [N, B]; view as [128, NCH, B] partition-major: DRAM row p*NCH+c.
    # Each partition's NCH rows are contiguous -> 8 KB store packets per stage.
    out_pcb = out.ap().rearrange("(p c) b -> p c b", p=128)

    from contextlib import ExitStack
    es = ExitStack()
    sb_ = lambda n, shape, dt=F32: es.enter_context(nc.sbuf_tensor(n, shape, dt))
    sem = lambda n: es.enter_context(nc.semaphore(n))
    g = sb_("g", [128, S, B], F16)
    ist = sb_("istb", [128, S], I32)
    wt = sb_("wt", [128, NCH * 16])
    opcf = sb_("opcf", [128, 4, NCH * 16])
    m3 = sb_("m3", [128, NCH * 16])
    ssum = sb_("ssum", [128, NCH]); rinv = sb_("rinv", [128, NCH])
    ckn = sb_("ckn", [128, NCH]); ck = sb_("ck", [128, 4, NCH])
    uf = sb_("uf", [128, 3, QC, B], F16)
    pf = sb_("pf", [128, 3, QC, B], F16)
    ob = sb_("ob", [128, 3, QC, B], F16)
    wf = sb_("wf", [128, NPOOL, B], F16)

    ldi = sem("ldi"); ldwt = sem("ldwt"); ldop = sem("ldop")
    gsg = [sem(f"gs{q}") for q in range(NQ)]
    cfA = sem("cfA"); cfE = sem("cfE"); ua = sem("ua")
    adV = sem("adV"); adP = sem("adP")
    pmd = sem("pmd"); pwE = sem("pwE"); ccs = sem("ccs")
    ods = [sem(f"od{s}") for s in range(NQ)]
    NDVE = NCH - NPOOL          # chunks 0..NDVE-1 final-combined on DVE

    def ob_free_wait(eng, q):
        # ob[q%3] free once the store of stage q-3 completed
        if q >= 3:
            eng.wait_ge(ods[q - 3], 16)

    def wait_ad(eng, n):
        # wait until the first n chunk-combines are done (all on DVE, in order)
        if n > 0:
            eng.wait_ge(adV, n)

    with es, nc.Block() as block:

        @block.sync
        def _(sync):
            sync.dma_start(ist[:], ist_in[:]).then_inc(ldi, 16)
            sync.dma_start(wt[:], wc_in[:]).then_inc(ldwt, 16)
            sync.dma_start(
                opcf.ap().rearrange("p a b -> p (a b)"), opc_in[:]
            ).then_inc(ldop, 16)
            for q in range(0, NQ, 2):   # even stages
                wait_ad(sync, 2 * (q + 1))
                sync.dma_start(
                    out_pcb[:, 2 * q:2 * q + 2, :], ob[:, q % 3]
                ).then_inc(ods[q], 16)

        @block.gpsimd
        def _(gp):
            gp.wait_ge(ldi, 16)
            for q, (lo, hi) in enumerate(groups):
                for s in range(lo, hi):
                    gp.indirect_dma_start(
                        out=g[:, s, :], out_offset=None, in_=xt[:],
                        in_offset=bass.IndirectOffsetOnAxis(
                            ap=ist[:, s:s + 1], axis=0),
                    ).then_inc(gsg[q], 16)


        @block.scalar
        def _(act):
            act.wait_ge(ldwt, 16)
            act.activation(wt[:], wt[:], EXP).then_inc(cfA, 1)
            act.wait_ge(cfE, 1)
            for q in range(NQ):
                if q >= 2 and (q - 1) % 2 == 1:   # store odd stage q-1
                    wait_ad(act, 2 * q)
                    act.dma_start(
                        out_pcb[:, 2 * (q - 1):2 * (q - 1) + 2, :],
                        ob[:, (q - 1) % 3],
                    ).then_inc(ods[q - 1], 16)
                act.wait_ge(gsg[q], 16 * (groups[q][1] - groups[q][0]))
                if q >= 3:
                    wait_ad(act, 2 * (q - 2))      # uf[q%3] free
                for j in range(QC):
                    c = q * QC + j
                    act.activation(                  # u = c3*b + c1
                        uf[:, q % 3, j, :], g[:, sb[c], :], IDENT,
                        bias=ck[:, 1, c:c + 1], scale=ck[:, 3, c:c + 1],
                    ).then_inc(ua, 1)
                for j in range(QC):
                    c = q * QC + j
                    if c >= NDVE:                    # w = c2*b + c0 for Pool
                        act.activation(
                            wf[:, c - NDVE, :], g[:, sb[c], :], IDENT,
                            bias=ck[:, 0, c:c + 1], scale=ck[:, 2, c:c + 1],
                        ).then_inc(pwE, 1)
            wait_ad(act, 2 * NQ)
            act.dma_start(
                out_pcb[:, 2 * (NQ - 1):2 * NQ, :], ob[:, (NQ - 1) % 3]
            ).then_inc(ods[NQ - 1], 16)

        @block.vector
        def _(vec):
            nedge = [0]

            def edge(inst):
                nedge[0] += 1
                inst.then_inc(ccs, 1)
                vec.wait_ge(ccs, nedge[0])

            vec.wait_ge(cfA, 1)
            e3 = wt.ap().rearrange("p (a b) -> p a b", b=16)
            m3r = m3.ap().rearrange("p (a b) -> p a b", b=16)
            edge(vec.reduce_sum(ssum[:], e3, axis=AX))
            edge(vec.reciprocal(rinv[:], ssum[:]))
            vec.wait_ge(ldop, 16)
            for k in range(4):
                o3c = opcf[:, k, :].rearrange("p (a b) -> p a b", b=16)
                edge(vec.tensor_mul(m3r, e3, o3c))
                edge(vec.reduce_sum(ckn[:], m3r, axis=AX))
                edge(vec.tensor_mul(ck[:, k, :], ckn[:], rinv[:]))
            vec.memset(m3[:, 0:1], 0.0).then_inc(cfE, 1)  # fence: ck committed
            for q in range(NQ):
                vec.wait_ge(ua, QC * (q + 1))
                for j in range(QC):
                    c = q * QC + j
                    vec.tensor_mul(                 # p = a*u
                        pf[:, q % 3, j, :], g[:, sa[c], :], uf[:, q % 3, j, :],
                    ).then_inc(pmd, 1)
                for j in range(QC):
                    c = q * QC + j
                    vec.wait_ge(pmd, c + 1)         # own mul drained (DVE RAW)
                    if j == 0:
                        ob_free_wait(vec, q)
                    if c < NDVE:
                        vec.affine_then_add(        # out = (b*c2 + c0) + p
                            ob[:, q % 3, j, :], g[:, sb[c], :],
                            pf[:, q % 3, j, :],
                            ck[:, 2, c:c + 1], ck[:, 0, c:c + 1],
                        ).then_inc(adV, 1)
                    else:                           # w from ACT: cheap f16 add
                        vec.wait_ge(pwE, c - NDVE + 1)
                        vec.tensor_add(
                            ob[:, q % 3, j, :], pf[:, q % 3, j, :],
                            wf[:, c - NDVE, :],
                        ).then_inc(adV, 1)

    nc.compile()
    return nc


def kernel(x, idx_a, idx_b, weights, trace=False):
    x = np.asarray(x, dtype=np.float32)
    weights = np.asarray(weights, dtype=np.float32)
    idx_a = np.asarray(idx_a).astype(np.int64)
    idx_b = np.asarray(idx_b).astype(np.int64)
    assert x.shape == (BATCH, IN_DIM) and weights.shape == (OUT_DIM, 16)

    key = (idx_a.tobytes(), idx_b.tobytes())
    if _cached.get("plan_key") != key:
        segs, slot_cols, perm, swap = _plan(idx_a, idx_b)
        _cached.update(plan_key=key, segs=segs, slot_cols=slot_cols,
                       perm=perm, swap=swap)
        if _cached.get("nc_segs") != tuple(segs):
            _cached["nc"] = build_nc(segs)
            _cached["nc_segs"] = tuple(segs)
    nc = _cached["nc"]
    segs, slot_cols = _cached["segs"], _cached["slot_cols"]
    perm, swap = _cached["perm"], _cached["swap"]

    xt = np.ascontiguousarray(x.astype(np.float16).T)  # [IN_DIM, B] fp16
    opc_row = np.repeat(_OP_COEFFS.T[:, None, :], NCH, axis=1).reshape(4 * NCH * 16)
    opc = np.ascontiguousarray(
        np.broadcast_to(opc_row[None, :], (128, 4 * NCH * 16))
    ).astype(np.float32)

    # effective per-neuron weights with a<->b swap permutation applied
    w_eff_all = np.where(swap.reshape(-1)[:, None],
                         weights[perm.reshape(-1)][:, _SWAP_PERM],
                         weights[perm.reshape(-1)])          # [NTRACK*NCH, 16]
    w_eff_all = w_eff_all.reshape(NCORES, 128, NCH, 16)

    in_maps = []
    for kcore in range(NCORES):
        tr = slice(kcore * 128, (kcore + 1) * 128)
        wc = np.ascontiguousarray(
            w_eff_all[kcore].reshape(128, NCH * 16)).astype(np.float32)
        in_maps.append({
            "xt": xt, "wc": wc, "opc": opc,
            "ist": np.ascontiguousarray(slot_cols[tr]).astype(np.int32),
        })
    res = run_bass_kernel_spmd(nc, in_maps, core_ids=list(range(NCORES)), trace=trace)

    buf = np.empty((OUT_DIM, B), dtype=np.float16)
    for kcore in range(NCORES):
        r = res.results[kcore]["out"].reshape(128, NCH, B)
        buf[perm[kcore * 128:(kcore + 1) * 128]] = r
    out = buf.T.astype(np.float32)
    kernel.last_exec_time_ns = res.exec_time_ns
    return out


kernel.last_exec_time_ns = None
